# revision 9
# baseline (speedup 1.0000x reference)
"""Multi-head causal attention (B=4, S=2048, D=1024, H=16) on 8 NeuronCores.

Sharding: core i handles batch b=i//2 and head-group g=i%2 (8 of 16 heads).
Tensor-parallel over heads: W_q/W_k/W_v column-sliced, W_o row-sliced; the
all-reduce after W_o is a host-side sum of the two partial outputs per batch.

Per-core kernel (S=2048, E=512 local width, 8 local heads of d_head=64):
  - QKV projections in fp32r (full-rate fp32 matmul mode), PSUM-accumulated
    over 8 K-chunks of 128.
  - Scores computed transposed (scoresT[k, q] = K_h Q_h^T) so softmax's
    denominator reduction lands on the PE via a ones-column appended to V.
  - No max-subtraction: |scores| <= ~2 here, exp is safe in fp32.
  - Causal masking via multiplicative {0,1} bf16 masks on the 4 diagonal
    chunk offsets; fully-masked chunks are skipped entirely.
  - exp on ScalarE (psum f32 -> sbuf bf16), attn@V in bf16 with a 65th
    lhsT column of ones producing softmax denominators in psum row 64.
  - Normalization: DVE reciprocal + gpsimd partition_broadcast + DVE mult.
  - W_o projection in fp32r from the transposed context.
Work is interleaved per 512-wide query window so PE (matmul) and ScalarE
(exp) stay concurrently busy.
"""

import numpy as np

import concourse.bass as bass
import concourse.tile as tile
from concourse import bacc, bass_utils, mybir

F32 = mybir.dt.float32
F32R = mybir.dt.float32r
BF16 = mybir.dt.bfloat16
AF = mybir.ActivationFunctionType

B = 4
S = 2048
D = 1024
NH_TOTAL = 16
DH = 64
E = 512          # local e-width (8 heads x 64)
NW = 4           # 512-wide query windows
WQ = 512
NDC = 8          # 128-wide d-model chunks
NEC = 4          # 128-wide local-e chunks (head pairs)
N_CORES = 8

_cache = {}


def _r(ap):
    return ap.bitcast(F32R)


def build_program():
    nc = bacc.Bacc(trn_type="TRN2", target_bir_lowering=False, debug=False)
    xt = nc.dram_tensor("xt", [D, S], F32R, kind="ExternalInput").ap()
    wq = nc.dram_tensor("wq", [D, E], F32R, kind="ExternalInput").ap()
    wk = nc.dram_tensor("wk", [D, E], F32R, kind="ExternalInput").ap()
    wv = nc.dram_tensor("wv", [D, E], F32R, kind="ExternalInput").ap()
    wo = nc.dram_tensor("wo", [E, D], F32R, kind="ExternalInput").ap()
    bq = nc.dram_tensor("bq", [E], F32, kind="ExternalInput").ap()
    bk = nc.dram_tensor("bk", [E], F32, kind="ExternalInput").ap()
    mk = nc.dram_tensor("mk", [128, 4 * 1024], BF16, kind="ExternalInput").ap()
    out = nc.dram_tensor("out", [S, D], F32, kind="ExternalOutput").ap()

    with tile.TileContext(nc) as tc:
        with (
            tc.tile_pool(name="const", bufs=1) as constp,
            tc.tile_pool(name="persist", bufs=1) as pers,
            tc.tile_pool(name="xtp", bufs=12) as xtp,
            tc.tile_pool(name="qtp", bufs=8) as qtp,
            tc.tile_pool(name="ctxp", bufs=8) as ctxp,
            tc.tile_pool(name="expp", bufs=3) as expp,
            tc.tile_pool(name="rcp", bufs=2) as rcp,
            tc.tile_pool(name="rbp", bufs=2) as rbp,
            tc.tile_pool(name="outp", bufs=4) as outp,
            tc.tile_pool(name="pp", bufs=2, space="PSUM") as pp,
            tc.tile_pool(name="psp", bufs=2, space="PSUM") as psp,
            tc.tile_pool(name="pcp", bufs=1, space="PSUM") as pcp,
        ):
            # ---- constants / persistent tensors ----
            wq_s = constp.tile([128, NDC, E], F32R, name="wq_s")
            wk_s = constp.tile([128, NDC, E], F32R, name="wk_s")
            wv_s = constp.tile([128, NDC, E], F32R, name="wv_s")
            for dc in range(NDC):
                nc.sync.dma_start(wq_s[:, dc, :], wq[dc * 128 : (dc + 1) * 128, :])
                nc.sync.dma_start(wk_s[:, dc, :], wk[dc * 128 : (dc + 1) * 128, :])
                nc.sync.dma_start(wv_s[:, dc, :], wv[dc * 128 : (dc + 1) * 128, :])
            wo_s = constp.tile([128, NEC, D], F32R, name="wo_s")
            for ec in range(NEC):
                nc.sync.dma_start(wo_s[:, ec, :], wo[ec * 128 : (ec + 1) * 128, :])
            bq_s = constp.tile([128, NEC], F32, name="bq_s")
            nc.sync.dma_start(bq_s[:], bq.rearrange("(c p) -> p c", p=128))
            bk_s = constp.tile([128, NEC], F32, name="bk_s")
            nc.sync.dma_start(bk_s[:], bk.rearrange("(c p) -> p c", p=128))
            mask_s = constp.tile([128, 4, 1024], BF16, name="mask_s")
            nc.sync.dma_start(mask_s[:], mk.rearrange("p (r j) -> p r j", r=4))
            ones_t = constp.tile([128, 64], F32, name="ones_t")
            nc.gpsimd.memset(ones_t[:], 1.0)

            # K^T [e, s] and V(+ones) [s, 8*(64+1)] persistent, bf16
            KT = pers.tile([128, NEC, S], BF16, name="KT")
            VP = pers.tile([128, 16, 520], BF16, name="VP")
            for sc16 in range(16):
                ones_ap = VP[:, sc16, :].rearrange("p (h c) -> p h c", h=8)[:, :, 64:65]
                nc.gpsimd.memset(ones_ap, 1.0)

            for w in range(NW):
                # ---- x^T chunks for this window ----
                xts = []
                for dc in range(NDC):
                    xtt = xtp.tile([128, WQ], F32R, name=f"xt_{w}_{dc}", tag="xt")
                    nc.sync.dma_start(
                        xtt[:], xt[dc * 128 : (dc + 1) * 128, w * WQ : (w + 1) * WQ]
                    )
                    xts.append(xtt)

                # ---- Q^T window ----
                qts = []
                for ec in range(NEC):
                    p = pp.tile([128, WQ], F32, name=f"pq_{w}_{ec}", tag="pp")
                    for dc in range(NDC):
                        nc.tensor.matmul(
                            p[:],
                            wq_s[:, dc, ec * 128 : (ec + 1) * 128],
                            xts[dc][:],
                            start=(dc == 0),
                            stop=(dc == NDC - 1),
                        )
                    qt = qtp.tile([128, WQ], BF16, name=f"qt_{w}_{ec}", tag="qt")
                    nc.vector.tensor_scalar_add(qt[:], p[:], bq_s[:, ec : ec + 1])
                    qts.append(qt)

                # ---- K^T window (into persistent KT) ----
                for ec in range(NEC):
                    p = pp.tile([128, WQ], F32, name=f"pk_{w}_{ec}", tag="pp")
                    for dc in range(NDC):
                        nc.tensor.matmul(
                            p[:],
                            wk_s[:, dc, ec * 128 : (ec + 1) * 128],
                            xts[dc][:],
                            start=(dc == 0),
                            stop=(dc == NDC - 1),
                        )
                    nc.vector.tensor_scalar_add(
                        KT[:, ec, w * WQ : (w + 1) * WQ], p[:], bk_s[:, ec : ec + 1]
                    )

                # ---- V window (into persistent VP, no bias: folded into b_o) ----
                for sc in range(4):
                    p = pp.tile([128, WQ], F32, name=f"pv_{w}_{sc}", tag="pp")
                    for dc in range(NDC):
                        nc.tensor.matmul(
                            p[:],
                            xts[dc][:, sc * 128 : (sc + 1) * 128],
                            wv_s[:, dc, :],
                            start=(dc == 0),
                            stop=(dc == NDC - 1),
                        )
                    sc16 = w * 4 + sc
                    vdst = VP[:, sc16, :].rearrange("p (h c) -> p h c", h=8)[:, :, 0:64]
                    nc.vector.tensor_copy(
                        vdst, p[:].rearrange("p (h c) -> p h c", h=8)
                    )

                # ---- attention for query window w ----
                ctx_w = []
                nkc = 4 * w + 4
                for hp in range(NEC):
                    c0 = pcp.tile([128, WQ], F32, name=f"c0_{w}_{hp}", tag="c0")
                    c1 = pcp.tile([128, WQ], F32, name=f"c1_{w}_{hp}", tag="c1")
                    for kc in range(nkc):
                        sp = psp.tile(
                            [128, 2 * WQ], F32, name=f"sp_{w}_{hp}_{kc}", tag="sp"
                        )
                        nc.tensor.matmul(
                            sp[:, 0:WQ],
                            KT[0:64, hp, kc * 128 : (kc + 1) * 128],
                            qts[hp][0:64, :],
                            start=True,
                            stop=True,
                        )
                        nc.tensor.matmul(
                            sp[:, WQ : 2 * WQ],
                            KT[64:128, hp, kc * 128 : (kc + 1) * 128],
                            qts[hp][64:128, :],
                            start=True,
                            stop=True,
                        )
                        ex = expp.tile(
                            [128, 2 * WQ], BF16, name=f"ex_{w}_{hp}_{kc}", tag="ex"
                        )
                        nc.scalar.activation(ex[:], sp[:], AF.Exp)
                        if kc >= 4 * w:
                            r = kc - 4 * w
                            wd = (r + 1) * 128
                            nc.vector.tensor_mul(
                                ex[:, 0:wd], ex[:, 0:wd], mask_s[:, r, 0:wd]
                            )
                            nc.vector.tensor_mul(
                                ex[:, WQ : WQ + wd],
                                ex[:, WQ : WQ + wd],
                                mask_s[:, r, WQ : WQ + wd],
                            )
                        nc.tensor.matmul(
                            c0[0:65, :],
                            VP[:, kc, (2 * hp) * 65 : (2 * hp) * 65 + 65],
                            ex[:, 0:WQ],
                            start=(kc == 0),
                            stop=(kc == nkc - 1),
                        )
                        nc.tensor.matmul(
                            c1[0:65, :],
                            VP[:, kc, (2 * hp + 1) * 65 : (2 * hp + 1) * 65 + 65],
                            ex[:, WQ : 2 * WQ],
                            start=(kc == 0),
                            stop=(kc == nkc - 1),
                        )
                    rc0 = rcp.tile([65, WQ], F32, name=f"rc0_{w}_{hp}", tag="rc0")
                    rc1 = rcp.tile([65, WQ], F32, name=f"rc1_{w}_{hp}", tag="rc1")
                    nc.vector.reciprocal(rc0[64:65, :], c0[64:65, :])
                    nc.vector.reciprocal(rc1[64:65, :], c1[64:65, :])
                    # broadcast 1/denom across partitions via K=1 matmuls into
                    # one psum bank: h0 -> rows 0:64, h1 -> rows 64:128
                    pb = pp.tile([128, WQ], F32, name=f"pb_{w}_{hp}", tag="pp")
                    nc.tensor.matmul(
                        pb[0:64, :], ones_t[64:65, 0:64], rc0[64:65, :],
                        start=True, stop=True,
                    )
                    nc.tensor.matmul(
                        pb[64:128, :], ones_t[64:65, 0:64], rc1[64:65, :],
                        start=True, stop=True,
                    )
                    rb = rbp.tile([128, WQ], F32, name=f"rb_{w}_{hp}", tag="rb")
                    nc.vector.tensor_copy(rb[:], pb[:])
                    ct = ctxp.tile([128, WQ], F32R, name=f"ct_{w}_{hp}", tag="ctx")
                    # raw h1 ctx: psum -> sbuf (DVE), partition-shift via DMA,
                    # then normalize each half partition-aligned
                    t1 = rcp.tile([64, WQ], F32R, name=f"t1_{w}_{hp}", tag="t1")
                    nc.vector.tensor_copy(t1[:], c1[0:64, :])
                    nc.sync.dma_start(ct[64:128, :], t1[:])
                    nc.vector.tensor_mul(ct[0:64, :], c0[0:64, :], rb[0:64, :])
                    nc.vector.tensor_mul(ct[64:128, :], ct[64:128, :], rb[64:128, :])
                    ctx_w.append(ct)

                # ---- W_o projection for this window's rows ----
                for sc in range(4):
                    for n2 in range(2):
                        p = pp.tile([128, WQ], F32, name=f"po_{w}_{sc}_{n2}", tag="pp")
                        for ec in range(NEC):
                            nc.tensor.matmul(
                                p[:],
                                ctx_w[ec][:, sc * 128 : (sc + 1) * 128],
                                wo_s[:, ec, n2 * WQ : (n2 + 1) * WQ],
                                start=(ec == 0),
                                stop=(ec == NEC - 1),
                            )
                        ot = outp.tile([128, WQ], F32, name=f"ot_{w}_{sc}_{n2}", tag="ot")
                        nc.vector.tensor_copy(ot[:], p[:])
                        r0 = w * WQ + sc * 128
                        nc.sync.dma_start(
                            out[r0 : r0 + 128, n2 * WQ : (n2 + 1) * WQ], ot[:]
                        )
    nc.compile()
    return nc


def _causal_masks():
    """4 diagonal-offset masks, [128, 4*1024] bf16, head-pair duplicated."""
    import ml_dtypes

    i = np.arange(128)[:, None]
    j = np.arange(WQ)[None, :]
    blocks = []
    for r in range(4):
        m = (j >= (i + r * 128)).astype(np.float32)
        blocks.append(np.concatenate([m, m], axis=1))
    return np.concatenate(blocks, axis=1).astype(ml_dtypes.bfloat16)


def make_in_maps(x, W_q, b_q, W_k, b_k, W_v, b_v, W_o, b_o):
    mask = _causal_masks()
    scale = 1.0 / np.sqrt(DH)
    in_maps = []
    for core in range(N_CORES):
        b, g = core // 2, core % 2
        sl = slice(g * E, (g + 1) * E)
        in_maps.append(
            {
                "xt": np.ascontiguousarray(x[b].T),
                "wq": np.ascontiguousarray(W_q[:, sl]) * np.float32(scale),
                "wk": np.ascontiguousarray(W_k[:, sl]),
                "wv": np.ascontiguousarray(W_v[:, sl]),
                "wo": np.ascontiguousarray(W_o[sl, :]),
                "bq": np.ascontiguousarray(b_q[sl]) * np.float32(scale),
                "bk": np.ascontiguousarray(b_k[sl]),
                "mk": mask,
            }
        )
    return in_maps


def assemble(results, W_o, b_v, b_o):
    bo_eff = (b_o + b_v @ W_o).astype(np.float32)
    out = np.empty((B, S, D), dtype=np.float32)
    for b in range(B):
        out[b] = results[2 * b]["out"] + results[2 * b + 1]["out"] + bo_eff
    return out


def kernel(x, W_q, b_q, W_k, b_k, W_v, b_v, W_o, b_o, _trace=False):
    x = np.asarray(x, dtype=np.float32)
    W_q = np.asarray(W_q, dtype=np.float32)
    b_q = np.asarray(b_q, dtype=np.float32)
    W_k = np.asarray(W_k, dtype=np.float32)
    b_k = np.asarray(b_k, dtype=np.float32)
    W_v = np.asarray(W_v, dtype=np.float32)
    b_v = np.asarray(b_v, dtype=np.float32)
    W_o = np.asarray(W_o, dtype=np.float32)
    b_o = np.asarray(b_o, dtype=np.float32)

    if "nc" not in _cache:
        _cache["nc"] = build_program()
    nc = _cache["nc"]
    in_maps = make_in_maps(x, W_q, b_q, W_k, b_k, W_v, b_v, W_o, b_o)
    res = bass_utils.run_bass_kernel_spmd(
        nc, in_maps, core_ids=list(range(N_CORES)), trace=_trace
    )
    out = assemble(res.results, W_o, b_v, b_o)
    if _trace:
        return out, res
    return out


# revision 11
# speedup vs baseline: 1.2438x; 1.2438x over previous
"""Multi-head causal attention (B=4, S=2048, D=1024, H=16) on 8 NeuronCores.

Sharding: core i handles batch b=i//2 and head-group g=i%2 (8 of 16 heads).
Tensor-parallel over heads: W_q/W_k/W_v column-sliced, W_o row-sliced; the
all-reduce after W_o is a host-side sum of the two partial outputs per batch.

Per-core kernel (S=2048, E=512 local width, 8 local heads of d_head=64):
  - QKV projections in fp32r (full-rate fp32 matmul mode), PSUM-accumulated
    over 8 K-chunks of 128.
  - Scores computed transposed (scoresT[k, q] = K_h Q_h^T) so softmax's
    denominator reduction lands on the PE via a ones-column appended to V.
  - No max-subtraction: |scores| <= ~2 here, exp is safe in fp32.
  - Causal masking via multiplicative {0,1} bf16 masks on the 4 diagonal
    chunk offsets; fully-masked chunks are skipped entirely.
  - exp on ScalarE (psum f32 -> sbuf bf16), attn@V in bf16 with a 65th
    lhsT column of ones producing softmax denominators in psum row 64.
  - Normalization: DVE reciprocal + gpsimd partition_broadcast + DVE mult.
  - W_o projection in fp32r from the transposed context.
Work is interleaved per 512-wide query window so PE (matmul) and ScalarE
(exp) stay concurrently busy.
"""

import numpy as np

import concourse.bass as bass
import concourse.tile as tile
from concourse import bacc, bass_utils, mybir

F32 = mybir.dt.float32
F32R = mybir.dt.float32r
BF16 = mybir.dt.bfloat16
AF = mybir.ActivationFunctionType

B = 4
S = 2048
D = 1024
NH_TOTAL = 16
DH = 64
E = 512          # local e-width (8 heads x 64)
NW = 4           # 512-wide query windows
WQ = 512
NDC = 8          # 128-wide d-model chunks
NEC = 4          # 128-wide local-e chunks (head pairs)
N_CORES = 8

_cache = {}


def _r(ap):
    return ap.bitcast(F32R)


def build_program():
    nc = bacc.Bacc(trn_type="TRN2", target_bir_lowering=False, debug=False)
    xt = nc.dram_tensor("xt", [D, S], F32R, kind="ExternalInput").ap()
    wq = nc.dram_tensor("wq", [D, E], F32R, kind="ExternalInput").ap()
    wk = nc.dram_tensor("wk", [D, E], F32R, kind="ExternalInput").ap()
    wv = nc.dram_tensor("wv", [D, E], F32R, kind="ExternalInput").ap()
    wo = nc.dram_tensor("wo", [E, D], F32R, kind="ExternalInput").ap()
    bq = nc.dram_tensor("bq", [E], F32, kind="ExternalInput").ap()
    bk = nc.dram_tensor("bk", [E], F32, kind="ExternalInput").ap()
    mk = nc.dram_tensor("mk", [128, 4 * 1024], BF16, kind="ExternalInput").ap()
    out = nc.dram_tensor("out", [S, D], F32, kind="ExternalOutput").ap()

    with tile.TileContext(nc) as tc:
        with (
            tc.tile_pool(name="const", bufs=1) as constp,
            tc.tile_pool(name="persist", bufs=1) as pers,
            tc.tile_pool(name="xtp", bufs=12) as xtp,
            tc.tile_pool(name="qtp", bufs=8) as qtp,
            tc.tile_pool(name="ctxp", bufs=8) as ctxp,
            tc.tile_pool(name="expp", bufs=3) as expp,
            tc.tile_pool(name="rcp", bufs=2) as rcp,
            tc.tile_pool(name="rbp", bufs=2) as rbp,
            tc.tile_pool(name="outp", bufs=4) as outp,
            tc.tile_pool(name="pp", bufs=2, space="PSUM") as pp,
            tc.tile_pool(name="psp", bufs=2, space="PSUM") as psp,
            tc.tile_pool(name="pcp", bufs=1, space="PSUM") as pcp,
        ):
            # ---- constants / persistent tensors ----
            wq_s = constp.tile([128, NDC, E], F32R, name="wq_s")
            wk_s = constp.tile([128, NDC, E], F32R, name="wk_s")
            wv_s = constp.tile([128, NDC, E], F32R, name="wv_s")
            for dc in range(NDC):
                nc.sync.dma_start(wq_s[:, dc, :], wq[dc * 128 : (dc + 1) * 128, :])
                nc.sync.dma_start(wk_s[:, dc, :], wk[dc * 128 : (dc + 1) * 128, :])
                nc.sync.dma_start(wv_s[:, dc, :], wv[dc * 128 : (dc + 1) * 128, :])
            wo_s = constp.tile([128, NEC, D], F32R, name="wo_s")
            for ec in range(NEC):
                nc.sync.dma_start(wo_s[:, ec, :], wo[ec * 128 : (ec + 1) * 128, :])
            bq_s = constp.tile([128, NEC], F32, name="bq_s")
            nc.sync.dma_start(bq_s[:], bq.rearrange("(c p) -> p c", p=128))
            bk_s = constp.tile([128, NEC], F32, name="bk_s")
            nc.sync.dma_start(bk_s[:], bk.rearrange("(c p) -> p c", p=128))
            mask_s = constp.tile([128, 4, 1024], BF16, name="mask_s")
            nc.sync.dma_start(mask_s[:], mk.rearrange("p (r j) -> p r j", r=4))
            ones_t = constp.tile([128, 64], F32, name="ones_t")
            nc.gpsimd.memset(ones_t[:], 1.0)

            # K^T [e, s] and V(+ones) [s, 8*(64+1)] persistent, bf16
            KT = pers.tile([128, NEC, S], BF16, name="KT")
            VP = pers.tile([128, 16, 520], BF16, name="VP")
            for sc16 in range(16):
                ones_ap = VP[:, sc16, :].rearrange("p (h c) -> p h c", h=8)[:, :, 64:65]
                nc.gpsimd.memset(ones_ap, 1.0)

            for w in range(NW):
                # ---- x^T chunks for this window ----
                xts = []
                for dc in range(NDC):
                    xtt = xtp.tile([128, WQ], F32R, name=f"xt_{w}_{dc}", tag="xt")
                    nc.sync.dma_start(
                        xtt[:], xt[dc * 128 : (dc + 1) * 128, w * WQ : (w + 1) * WQ]
                    )
                    xts.append(xtt)

                # ---- Q^T window ----
                qts = []
                for ec in range(NEC):
                    p = pp.tile([128, WQ], F32, name=f"pq_{w}_{ec}", tag="pp")
                    for dc in range(NDC):
                        nc.tensor.matmul(
                            p[:],
                            wq_s[:, dc, ec * 128 : (ec + 1) * 128],
                            xts[dc][:],
                            start=(dc == 0),
                            stop=(dc == NDC - 1),
                        )
                    qt = qtp.tile([128, WQ], BF16, name=f"qt_{w}_{ec}", tag="qt")
                    nc.vector.tensor_scalar_add(qt[:], p[:], bq_s[:, ec : ec + 1])
                    qts.append(qt)

                # ---- K^T window (into persistent KT) ----
                for ec in range(NEC):
                    p = pp.tile([128, WQ], F32, name=f"pk_{w}_{ec}", tag="pp")
                    for dc in range(NDC):
                        nc.tensor.matmul(
                            p[:],
                            wk_s[:, dc, ec * 128 : (ec + 1) * 128],
                            xts[dc][:],
                            start=(dc == 0),
                            stop=(dc == NDC - 1),
                        )
                    nc.vector.tensor_scalar_add(
                        KT[:, ec, w * WQ : (w + 1) * WQ], p[:], bk_s[:, ec : ec + 1]
                    )

                # ---- V window (into persistent VP, no bias: folded into b_o) ----
                for sc in range(4):
                    p = pp.tile([128, WQ], F32, name=f"pv_{w}_{sc}", tag="pp")
                    for dc in range(NDC):
                        nc.tensor.matmul(
                            p[:],
                            xts[dc][:, sc * 128 : (sc + 1) * 128],
                            wv_s[:, dc, :],
                            start=(dc == 0),
                            stop=(dc == NDC - 1),
                        )
                    sc16 = w * 4 + sc
                    vdst = VP[:, sc16, :].rearrange("p (h c) -> p h c", h=8)[:, :, 0:64]
                    nc.vector.tensor_copy(
                        vdst, p[:].rearrange("p (h c) -> p h c", h=8)
                    )

                # ---- attention for query window w ----
                ctx_w = []
                nkc = 4 * w + 4
                for hp in range(NEC):
                    c0 = pcp.tile([128, WQ], F32, name=f"c0_{w}_{hp}", tag="c0")
                    c1 = pcp.tile([128, WQ], F32, name=f"c1_{w}_{hp}", tag="c1")
                    for kc in range(nkc):
                        sp = psp.tile(
                            [128, 2 * WQ], F32, name=f"sp_{w}_{hp}_{kc}", tag="sp"
                        )
                        nc.tensor.matmul(
                            sp[:, 0:WQ],
                            KT[0:64, hp, kc * 128 : (kc + 1) * 128],
                            qts[hp][0:64, :],
                            start=True,
                            stop=True,
                        )
                        nc.tensor.matmul(
                            sp[:, WQ : 2 * WQ],
                            KT[64:128, hp, kc * 128 : (kc + 1) * 128],
                            qts[hp][64:128, :],
                            start=True,
                            stop=True,
                        )
                        ex = expp.tile(
                            [128, 2 * WQ], BF16, name=f"ex_{w}_{hp}_{kc}", tag="ex"
                        )
                        nc.scalar.activation(ex[:], sp[:], AF.Exp)
                        if kc >= 4 * w:
                            r = kc - 4 * w
                            wd = (r + 1) * 128
                            nc.vector.tensor_mul(
                                ex[:, 0:wd], ex[:, 0:wd], mask_s[:, r, 0:wd]
                            )
                            nc.vector.tensor_mul(
                                ex[:, WQ : WQ + wd],
                                ex[:, WQ : WQ + wd],
                                mask_s[:, r, WQ : WQ + wd],
                            )
                        nc.tensor.matmul(
                            c0[0:65, :],
                            VP[:, kc, (2 * hp) * 65 : (2 * hp) * 65 + 65],
                            ex[:, 0:WQ],
                            start=(kc == 0),
                            stop=(kc == nkc - 1),
                        )
                        nc.tensor.matmul(
                            c1[0:65, :],
                            VP[:, kc, (2 * hp + 1) * 65 : (2 * hp + 1) * 65 + 65],
                            ex[:, WQ : 2 * WQ],
                            start=(kc == 0),
                            stop=(kc == nkc - 1),
                        )
                    # Evacuate raw ctx+denom to SBUF right away (frees the
                    # psum banks so the next head-pair's AV can start).
                    cr0 = rcp.tile([65, WQ], F32, name=f"cr0_{w}_{hp}", tag="cr0")
                    nc.vector.tensor_copy(cr0[:], c0[0:65, :])
                    cr1 = rcp.tile([65, WQ], F32, name=f"cr1_{w}_{hp}", tag="cr1")
                    nc.vector.tensor_copy(cr1[:], c1[0:65, :])
                    # broadcast denoms across partitions via K=1 matmuls into
                    # one psum bank: h0 -> rows 0:64, h1 -> rows 64:128
                    pb = pp.tile([128, WQ], F32, name=f"pb_{w}_{hp}", tag="pp")
                    nc.tensor.matmul(
                        pb[0:64, :], ones_t[64:65, 0:64], cr0[64:65, :],
                        start=True, stop=True,
                    )
                    nc.tensor.matmul(
                        pb[64:128, :], ones_t[64:65, 0:64], cr1[64:65, :],
                        start=True, stop=True,
                    )
                    rbw = rbp.tile([128, WQ], F32, name=f"rbw_{w}_{hp}", tag="rbw")
                    nc.vector.tensor_copy(rbw[:], pb[:])
                    rb = rbp.tile([128, WQ], F32, name=f"rb_{w}_{hp}", tag="rb")
                    nc.vector.reciprocal_approx_fast(rb[:], rbw[:])
                    ct = ctxp.tile([128, WQ], F32R, name=f"ct_{w}_{hp}", tag="ctx")
                    # h1 raw ctx -> ct rows 64:128 (partition-shift DMA), then
                    # normalize each half partition-aligned
                    nc.sync.dma_start(ct[64:128, :], cr1[0:64, :].bitcast(F32R))
                    nc.vector.tensor_mul(ct[0:64, :], cr0[0:64, :], rb[0:64, :])
                    nc.vector.tensor_mul(ct[64:128, :], ct[64:128, :], rb[64:128, :])
                    ctx_w.append(ct)

                # ---- W_o projection for this window's rows ----
                for sc in range(4):
                    for n2 in range(2):
                        p = pp.tile([128, WQ], F32, name=f"po_{w}_{sc}_{n2}", tag="pp")
                        for ec in range(NEC):
                            nc.tensor.matmul(
                                p[:],
                                ctx_w[ec][:, sc * 128 : (sc + 1) * 128],
                                wo_s[:, ec, n2 * WQ : (n2 + 1) * WQ],
                                start=(ec == 0),
                                stop=(ec == NEC - 1),
                            )
                        ot = outp.tile([128, WQ], F32, name=f"ot_{w}_{sc}_{n2}", tag="ot")
                        nc.vector.tensor_copy(ot[:], p[:])
                        r0 = w * WQ + sc * 128
                        nc.sync.dma_start(
                            out[r0 : r0 + 128, n2 * WQ : (n2 + 1) * WQ], ot[:]
                        )
    nc.compile()
    return nc


def _causal_masks():
    """4 diagonal-offset masks, [128, 4*1024] bf16, head-pair duplicated."""
    import ml_dtypes

    i = np.arange(128)[:, None]
    j = np.arange(WQ)[None, :]
    blocks = []
    for r in range(4):
        m = (j >= (i + r * 128)).astype(np.float32)
        blocks.append(np.concatenate([m, m], axis=1))
    return np.concatenate(blocks, axis=1).astype(ml_dtypes.bfloat16)


def make_in_maps(x, W_q, b_q, W_k, b_k, W_v, b_v, W_o, b_o):
    mask = _causal_masks()
    scale = 1.0 / np.sqrt(DH)
    in_maps = []
    for core in range(N_CORES):
        b, g = core // 2, core % 2
        sl = slice(g * E, (g + 1) * E)
        in_maps.append(
            {
                "xt": np.ascontiguousarray(x[b].T),
                "wq": np.ascontiguousarray(W_q[:, sl]) * np.float32(scale),
                "wk": np.ascontiguousarray(W_k[:, sl]),
                "wv": np.ascontiguousarray(W_v[:, sl]),
                "wo": np.ascontiguousarray(W_o[sl, :]),
                "bq": np.ascontiguousarray(b_q[sl]) * np.float32(scale),
                "bk": np.ascontiguousarray(b_k[sl]),
                "mk": mask,
            }
        )
    return in_maps


def assemble(results, W_o, b_v, b_o):
    bo_eff = (b_o + b_v @ W_o).astype(np.float32)
    out = np.empty((B, S, D), dtype=np.float32)
    for b in range(B):
        out[b] = results[2 * b]["out"] + results[2 * b + 1]["out"] + bo_eff
    return out


def kernel(x, W_q, b_q, W_k, b_k, W_v, b_v, W_o, b_o, _trace=False):
    x = np.asarray(x, dtype=np.float32)
    W_q = np.asarray(W_q, dtype=np.float32)
    b_q = np.asarray(b_q, dtype=np.float32)
    W_k = np.asarray(W_k, dtype=np.float32)
    b_k = np.asarray(b_k, dtype=np.float32)
    W_v = np.asarray(W_v, dtype=np.float32)
    b_v = np.asarray(b_v, dtype=np.float32)
    W_o = np.asarray(W_o, dtype=np.float32)
    b_o = np.asarray(b_o, dtype=np.float32)

    if "nc" not in _cache:
        _cache["nc"] = build_program()
    nc = _cache["nc"]
    in_maps = make_in_maps(x, W_q, b_q, W_k, b_k, W_v, b_v, W_o, b_o)
    res = bass_utils.run_bass_kernel_spmd(
        nc, in_maps, core_ids=list(range(N_CORES)), trace=_trace
    )
    out = assemble(res.results, W_o, b_v, b_o)
    if _trace:
        return out, res
    return out
